# revision 1
# baseline (speedup 1.0000x reference)
"""Trainium2 Bass kernel for nn_BitResidualBlock (dense_cnn).

Reference computation (per batch element, C=512 channels, T=4096):
    for d in (1, 3, 5):
        h = bitconv1d(x, w1, b1, dilation=d)     # ternary-quantized weights
        h = snake_beta(h, alpha, beta)           # x + sin(a*x)^2 / (b+eps)
        h = bitconv1d(h, w2, b2, dilation=1)
        x = x + h

Strategy:
  - Data-parallel over batch: 8 batch elements -> 8 NeuronCores, no
    collectives. Identical SPMD program, per-core input shard.
  - BitNet ternary quantization is done on HOST (it is a per-tensor
    scalar + ternarize): the ternary weights {-1,0,+1} are shipped as
    bf16 (exact), the scale s is applied in f32 on ScalarE.
  - Each conv = 12 accumulating 128x128x512 matmuls per output tile
    (4 ci chunks x 3 taps), bf16 operands, fp32 PSUM accumulate.
  - snake: z kept in f32; sin evaluated on ScalarE (LUT valid on
    [-pi, pi]) after range reduction mod pi using a f32->i32->f32
    round-trip (sin^2 is pi-periodic so any integer multiple works).
  - Residual x stays resident in SBUF in f32 across all 3 blocks.
"""

import numpy as np
import ml_dtypes

import concourse.bass as bass
import concourse.mybir as mybir
import concourse.tile as tile
from concourse.vector_clock import ScopedClock
from concourse.bass_utils import run_bass_kernel_spmd

AF = mybir.ActivationFunctionType
ALU = mybir.AluOpType
F32 = mybir.dt.float32
I32 = mybir.dt.int32
BF16 = mybir.dt.bfloat16

B, C, T, K = 8, 512, 4096, 3
DILATIONS = (1, 3, 5)
EPS_Q = 1e-5
EPS_SNAKE = 1e-9

P = 128          # partitions
NCH = C // P     # 4 channel chunks
TT = 512         # time-tile (one PSUM bank of f32)
NT = T // TT     # 8 time tiles
PAD = 8          # zero pad each side of bf16 activation tiles
TPW = T + 2 * PAD
NPARAM = 21      # 7 param columns per block x 3 blocks

# Set by the test harness to profile; kernel() records exec time here.
TRACE = False
LAST_EXEC_NS = None
LAST_RESULT = None


class SplitDrainTileContext(tile.TileContext):
    """TileContext whose tail drain splits its sem waits across
    single-wait instructions.

    The walrus build in this environment rejects a Drain carrying more
    than a couple of sync waits ("Too many sync wait commands",
    CoreV3GenImpl.cpp setupSyncWait). Absorb the outstanding vector-clock
    waits with one single-wait nop per semaphore before draining.
    """

    def _drain_and_barrier(self, tick_clock, wait_clock):
        collector = self.nc.sync.nop(nofuse=True)
        wait_clock.add_sem_waits(
            collector.ins, ScopedClock({None: tick_clock.global_clock})
        )
        si = collector.ins.sync_info
        waits = list(si.on_wait) if si is not None else []
        if len(waits) > 1:
            collector.ins.sync_info = mybir.SyncInfo(
                on_wait=waits[:1], on_update=list(si.on_update)
            )
            for w in waits[1:]:
                extra = self.nc.sync.nop(nofuse=True)
                extra.ins.sync_info = mybir.SyncInfo(on_wait=[w], on_update=[])
        self.nc.sync.drain()
        self.nc.all_engine_barrier()
        assert self.sems is not None
        popped = self.nc._tile_sem_poison_stack.pop()
        assert popped is self._sem_poison
        self.nc.clear_and_free_semaphores(list(self.sems.allocated().values()))
        self.nc.all_engine_barrier()


def _split_sync_waits(nc, maxw=1):
    """Walrus in this environment encodes at most one sync wait per
    instruction ("Too many sync wait commands" otherwise). Move excess
    waits onto single-wait EventSemaphore instructions inserted just
    before the owner on the same engine (engines run their stream in
    block order, so the waits still gate the instruction)."""
    for bb in nc.main_func.blocks:
        out = []
        changed = False
        for ins in bb.instructions:
            si = getattr(ins, "sync_info", None)
            if si is not None and len(si.on_wait) > maxw:
                waits = list(si.on_wait)
                extra, keep = waits[:-maxw], waits[-maxw:]
                for w in extra:
                    ev = mybir.InstEventSemaphore(
                        name=nc.get_next_instruction_name(), ins=[], outs=[])
                    ev.engine = ins.engine
                    ev.sync_info = mybir.SyncInfo(on_wait=[w], on_update=[])
                    nc.register_instruction(ev, overwrite=True)
                    out.append(ev)
                ins.sync_info = mybir.SyncInfo(
                    on_wait=keep, on_update=list(si.on_update))
                changed = True
            out.append(ins)
        if changed:
            bb.instructions = out


def build_nc():
    nc = bass.Bass(target_bir_lowering=False)
    x_d = nc.dram_tensor("x", [C, T], F32, kind="ExternalInput")
    xb16_d = nc.dram_tensor("xb16", [C, T], BF16, kind="ExternalInput")
    wt_d = nc.dram_tensor("wt", [3, 2, NCH, P, K * NCH * P], BF16,
                          kind="ExternalInput")
    pp_d = nc.dram_tensor("pp", [NCH, P, NPARAM], F32, kind="ExternalInput")
    y_d = nc.dram_tensor("y", [C, T], F32, kind="ExternalOutput")

    with SplitDrainTileContext(nc) as tc:
        with (
            tc.tile_pool(name="persist", bufs=1) as p1,
            tc.tile_pool(name="wts", bufs=1) as pw,
            tc.tile_pool(name="t2", bufs=2) as p2,
            tc.tile_pool(name="t3", bufs=3) as p3,
            tc.tile_pool(name="tz", bufs=3) as pz,
            tc.tile_pool(name="ps", bufs=6, space="PSUM") as pps,
        ):
            xf = [p1.tile([P, T], F32, tag=f"xf{c}", name=f"xf{c}") for c in range(NCH)]
            xb = [p1.tile([P, TPW], BF16, tag=f"xb{c}", name=f"xb{c}") for c in range(NCH)]
            hb = [p1.tile([P, TPW], BF16, tag=f"hb{c}", name=f"hb{c}") for c in range(NCH)]
            pt = [p1.tile([P, NPARAM], F32, tag=f"pt{c}", name=f"pt{c}") for c in range(NCH)]

            def alloc_w(i, conv):
                return [pw.tile([P, K * NCH * P], BF16,
                                tag=f"w{conv}_{c}", name=f"w{conv}_{i}_{c}")
                        for c in range(NCH)]

            def load_weights(i):
                w1t, w2t = alloc_w(i, 1), alloc_w(i, 2)
                for c in range(NCH):
                    nc.sync.dma_start(out=w1t[c], in_=wt_d[i, 0, c])
                for c in range(NCH):
                    nc.sync.dma_start(out=w2t[c], in_=wt_d[i, 1, c])
                return w1t, w2t

            for c in range(NCH):
                nc.sync.dma_start(out=pt[c], in_=pp_d[c])
                nc.vector.memset(xb[c][:, 0:PAD], 0.0)
                nc.vector.memset(xb[c][:, PAD + T:TPW], 0.0)
                nc.vector.memset(hb[c][:, 0:PAD], 0.0)
                nc.vector.memset(hb[c][:, PAD + T:TPW], 0.0)

            # All HWDGE DMAs drain through one FIFO queue at ~360 GB/s, so
            # the queue ORDER is the startup critical path. The first conv
            # matmuls need block-0 w1 + xb time-tiles 0..1; then w2; the
            # rest of xb; and last the f32 x (only needed from the conv2
            # epilogue, ~100us in). x is shipped pre-cast to bf16 by the
            # host so the critical bytes are halved and no on-chip cast
            # pass is needed.
            # Weights are co-major in the free dim, so the co=0 quarter of
            # w1 (the only weights the first 8 conv tiles need) is one
            # contiguous strip per ci chunk - land it plus xb jt0..1, then
            # the rest of w1, w2, the rest of xb, and last the f32 x.
            w1t0 = alloc_w(0, 1)
            CW = K * P
            for c in range(NCH):
                nc.sync.dma_start(out=w1t0[c][:, 0:CW],
                                  in_=wt_d[0, 0, c][:, 0:CW])
            for jt in range(4):
                for c in range(NCH):
                    sl = slice(jt * TT, (jt + 1) * TT)
                    nc.sync.dma_start(
                        out=xb[c][:, PAD + jt * TT:PAD + (jt + 1) * TT],
                        in_=xb16_d[c * P:(c + 1) * P, sl])
            for c in range(NCH):
                nc.sync.dma_start(out=w1t0[c][:, CW:],
                                  in_=wt_d[0, 0, c][:, CW:])
            for jt in range(4, NT):
                for c in range(NCH):
                    sl = slice(jt * TT, (jt + 1) * TT)
                    nc.sync.dma_start(
                        out=xb[c][:, PAD + jt * TT:PAD + (jt + 1) * TT],
                        in_=xb16_d[c * P:(c + 1) * P, sl])
            w2t0 = alloc_w(0, 2)
            for c in range(NCH):
                nc.sync.dma_start(out=w2t0[c], in_=wt_d[0, 1, c])
            for c in range(NCH):
                nc.sync.dma_start(out=xf[c], in_=x_d[c * P:(c + 1) * P, :])
            wcur = (w1t0, w2t0)

            for i in range(3):
                d = DILATIONS[i]
                base = i * 7
                w1t, w2t = wcur
                if i < 2:
                    wnext = load_weights(i + 1)

                # conv1 (dilation d) + snake -> hb (bf16, padded)
                for co in range(NCH):
                    b1ap = pt[co][:, base + 0:base + 1]
                    s1ap = pt[co][:, base + 1:base + 2]
                    raap = pt[co][:, base + 2:base + 3]
                    rbap = pt[co][:, base + 3:base + 4]
                    ibap = pt[co][:, base + 4:base + 5]
                    for jt in range(NT):
                        ps = pps.tile([P, TT], F32, tag="ps")
                        col0 = PAD + jt * TT
                        n = 0
                        for ci in range(NCH):
                            for k in range(K):
                                sh = (k - 1) * d
                                nc.tensor.matmul(
                                    ps,
                                    w1t[ci][:, (co * K + k) * P:
                                            (co * K + k + 1) * P],
                                    xb[ci][:, col0 + sh:col0 + sh + TT],
                                    start=(n == 0), stop=(n == 11),
                                )
                                n += 1
                        # z = s1*psum + b1 (the pre-activation, kept f32)
                        z = pz.tile([P, TT], F32, tag="z")
                        nc.scalar.activation(z, ps, AF.Identity,
                                             bias=b1ap, scale=s1ap)
                        # r = a*z/pi (folded: psum*(s1*a/pi) + b1*a/pi)
                        r = p3.tile([P, TT], F32, tag="r")
                        nc.scalar.activation(r, ps, AF.Identity,
                                             bias=rbap, scale=raap)
                        # range-reduce: dd = r - int(r)  (|dd| < 1)
                        ri = p2.tile([P, TT], I32, tag="ri")
                        nc.vector.tensor_copy(ri, r)
                        dd = p2.tile([P, TT], F32, tag="dd")
                        nc.vector.tensor_sub(dd, r, ri)
                        # u = sin(pi*dd) == +-sin(a*z);  u^2 is what we need
                        u = p3.tile([P, TT], F32, tag="u")
                        nc.scalar.activation(u, dd, AF.Sin,
                                             scale=float(np.pi))
                        v = p2.tile([P, TT], F32, tag="v")
                        nc.vector.tensor_mul(v, u, u)
                        # h = z + invb * u^2, cast to bf16 into padded hb
                        nc.vector.scalar_tensor_tensor(
                            hb[co][:, col0:col0 + TT], v, ibap, z,
                            ALU.mult, ALU.add,
                        )

                # conv2 (dilation 1) + residual add into xf
                for co in range(NCH):
                    b2ap = pt[co][:, base + 5:base + 6]
                    s2ap = pt[co][:, base + 6:base + 7]
                    for jt in range(NT):
                        ps = pps.tile([P, TT], F32, tag="ps")
                        col0 = PAD + jt * TT
                        n = 0
                        for ci in range(NCH):
                            for k in range(K):
                                sh = k - 1
                                nc.tensor.matmul(
                                    ps,
                                    w2t[ci][:, (co * K + k) * P:
                                            (co * K + k + 1) * P],
                                    hb[ci][:, col0 + sh:col0 + sh + TT],
                                    start=(n == 0), stop=(n == 11),
                                )
                                n += 1
                        t = p3.tile([P, TT], F32, tag="t")
                        nc.scalar.activation(t, ps, AF.Identity,
                                             bias=b2ap, scale=s2ap)
                        xsl = xf[co][:, jt * TT:(jt + 1) * TT]
                        nc.vector.tensor_add(xsl, xsl, t)
                        if i < 2:
                            nc.vector.tensor_copy(
                                xb[co][:, col0:col0 + TT], xsl)
                        else:
                            nc.sync.dma_start(
                                out=y_d[co * P:(co + 1) * P,
                                        jt * TT:(jt + 1) * TT],
                                in_=xsl)
                if i < 2:
                    wcur = wnext
    _split_sync_waits(nc)
    return nc


_NC = None


def _get_nc():
    global _NC
    if _NC is None:
        _NC = build_nc()
    return _NC


def _host_params(w1, b1, alpha, beta, w2, b2):
    """Ternarize weights and fold snake/scale params, matching the
    reference's jax-on-CPU float32 numerics."""
    import jax
    import jax.numpy as jnp

    cpu = jax.devices("cpu")[0]

    wt = np.empty((3, 2, NCH, P, K * NCH * P), dtype=ml_dtypes.bfloat16)
    pp = np.zeros((NCH, P, NPARAM), dtype=np.float32)
    pi = np.float32(np.pi)

    with jax.default_device(cpu):
        for i in range(3):
            svals = []
            for conv, w in ((0, w1[i]), (1, w2[i])):
                s = jnp.mean(jnp.abs(w))
                tern = jnp.clip(jnp.round(w / (s + EPS_Q)), -1.0, 1.0)
                svals.append(np.float32(s))
                tern = np.asarray(tern, dtype=np.float32)
                # [co, ci, k] -> [cich, ci_in, coch, k, co_in] (co-major
                # free dim so a single co chunk is one contiguous DMA)
                t5 = tern.reshape(NCH, P, NCH, P, K).transpose(2, 3, 0, 4, 1)
                wt[i, conv] = t5.reshape(NCH, P, K * NCH * P).astype(
                    ml_dtypes.bfloat16)
            s1, s2 = svals
            a = np.asarray(jnp.exp(alpha[i]), dtype=np.float32)
            bsn = np.asarray(jnp.exp(beta[i]), dtype=np.float32)
            invb = np.asarray(
                jnp.float32(1.0) / (jnp.asarray(bsn) + jnp.float32(EPS_SNAKE)),
                dtype=np.float32)
            base = i * 7
            pp[:, :, base + 0] = b1[i].reshape(NCH, P)
            pp[:, :, base + 1] = s1
            pp[:, :, base + 2] = (s1 * a / pi).reshape(NCH, P)
            pp[:, :, base + 3] = (b1[i] * a / pi).reshape(NCH, P)
            pp[:, :, base + 4] = invb.reshape(NCH, P)
            pp[:, :, base + 5] = b2[i].reshape(NCH, P)
            pp[:, :, base + 6] = s2
    return wt, pp


def kernel(x, w1, b1, alpha, beta, w2, b2):
    global LAST_EXEC_NS
    x = np.asarray(x, dtype=np.float32)
    w1 = np.asarray(w1, dtype=np.float32)
    b1 = np.asarray(b1, dtype=np.float32)
    alpha = np.asarray(alpha, dtype=np.float32)
    beta = np.asarray(beta, dtype=np.float32)
    w2 = np.asarray(w2, dtype=np.float32)
    b2 = np.asarray(b2, dtype=np.float32)

    wt, pp = _host_params(w1, b1, alpha, beta, w2, b2)
    nc = _get_nc()

    in_maps = [
        {"x": x[b], "xb16": x[b].astype(ml_dtypes.bfloat16),
         "wt": wt, "pp": pp}
        for b in range(B)
    ]
    res = run_bass_kernel_spmd(
        nc, in_maps, core_ids=list(range(B)), trace=TRACE)
    LAST_EXEC_NS = res.exec_time_ns
    global LAST_RESULT
    LAST_RESULT = res

    out = np.stack([res.results[b]["y"] for b in range(B)], axis=0)
    return out.astype(np.float32)



# revision 8
# speedup vs baseline: 1.1190x; 1.1190x over previous
"""Trainium2 Bass kernel for nn_BitResidualBlock (dense_cnn).

Reference computation (per batch element, C=512 channels, T=4096):
    for d in (1, 3, 5):
        h = bitconv1d(x, w1, b1, dilation=d)     # ternary-quantized weights
        h = snake_beta(h, alpha, beta)           # x + sin(a*x)^2 / (b+eps)
        h = bitconv1d(h, w2, b2, dilation=1)
        x = x + h

Strategy:
  - Data-parallel over batch: 8 batch elements -> 8 NeuronCores, no
    collectives. Identical SPMD program, per-core input shard.
  - conv1 (dilated): direct matmul form, 12 accumulating 128x128x512
    bf16 matmuls per output tile (4 ci chunks x 3 taps).
  - conv2 (dilation 1): Winograd F(2,3). h is kept as even/odd planes
    hE[u]=h[2u], hO[u]=h[2u+1]; four moving signals per u-tile
        X0 = O[u]-O[u+1], X1 = E+O', X2 = O'-E, X3 = E-E'
    feed 4 m-point matmuls with host-transformed stationary weights
        W0 = g0, W1 = (g0+g1+g2)/2, W2 = (g0-g1+g2)/2, W3 = g2
    (exact in bf16 since g is ternary). Outputs:
        y[2u]   = m0 + m1 + m2,   y[2u+1] = m1 - m2 - m3
    -> 16 matmuls per 1024 outputs instead of 24 (1.5x fewer PE cycles).
  - snake: z' = s1*s2*psum + s2*b1 (s2 pre-folded so conv2 weights stay
    exact integers); r = (s1*a/pi)*psum + b1*a/pi; dd = mod(r, 1)
    (sin^2 is pi-periodic so any integer shift works); u = sin(pi*dd)
    and u^2 on ScalarE; h' = s2*h = invb*s2*u^2 + z' written bf16 into
    the E/O planes.
  - Residual x stays resident in SBUF in f32 across all 3 blocks.
"""

import numpy as np
import ml_dtypes

import concourse.bass as bass
import concourse.mybir as mybir
import concourse.tile as tile
from concourse.vector_clock import ScopedClock
from concourse.bass_utils import run_bass_kernel_spmd

AF = mybir.ActivationFunctionType
ALU = mybir.AluOpType
F32 = mybir.dt.float32
I32 = mybir.dt.int32
BF16 = mybir.dt.bfloat16

B, C, T, K = 8, 512, 4096, 3
DILATIONS = (1, 3, 5)
EPS_Q = 1e-5
EPS_SNAKE = 1e-9

P = 128          # partitions
NCH = C // P     # 4 channel chunks
TT = 512         # conv1 time-tile (one PSUM bank of f32)
NT = T // TT     # 8 conv1 time tiles
PAD = 8          # zero pad each side of bf16 activation tiles
TPW = T + 2 * PAD
U = T // 2       # winograd u positions
UT = 512         # u-tile (one PSUM bank)
NU = U // UT     # 4 u tiles
PADU = 4         # zero pad each side of h planes (in u units)
UPW = U + 2 * PADU
NPARAM = 18      # 6 param columns per block x 3 blocks

# Set by the test harness to profile; kernel() records exec time here.
TRACE = False
LAST_EXEC_NS = None
LAST_RESULT = None


class SplitDrainTileContext(tile.TileContext):
    """TileContext whose tail drain splits its sem waits across
    single-wait instructions.

    The walrus build in this environment rejects a Drain carrying more
    than a couple of sync waits ("Too many sync wait commands",
    CoreV3GenImpl.cpp setupSyncWait). Absorb the outstanding vector-clock
    waits with one single-wait nop per semaphore before draining.
    """

    def _drain_and_barrier(self, tick_clock, wait_clock):
        collector = self.nc.sync.nop(nofuse=True)
        wait_clock.add_sem_waits(
            collector.ins, ScopedClock({None: tick_clock.global_clock})
        )
        si = collector.ins.sync_info
        waits = list(si.on_wait) if si is not None else []
        if len(waits) > 1:
            collector.ins.sync_info = mybir.SyncInfo(
                on_wait=waits[:1], on_update=list(si.on_update)
            )
            for w in waits[1:]:
                extra = self.nc.sync.nop(nofuse=True)
                extra.ins.sync_info = mybir.SyncInfo(on_wait=[w], on_update=[])
        self.nc.sync.drain()
        self.nc.all_engine_barrier()
        assert self.sems is not None
        popped = self.nc._tile_sem_poison_stack.pop()
        assert popped is self._sem_poison
        self.nc.clear_and_free_semaphores(list(self.sems.allocated().values()))
        self.nc.all_engine_barrier()


def _split_sync_waits(nc, maxw=1):
    """Walrus in this environment encodes at most one sync wait per
    instruction ("Too many sync wait commands" otherwise). Move excess
    waits onto single-wait EventSemaphore instructions inserted just
    before the owner on the same engine (engines run their stream in
    block order, so the waits still gate the instruction)."""
    for bb in nc.main_func.blocks:
        out = []
        changed = False
        for ins in bb.instructions:
            si = getattr(ins, "sync_info", None)
            if si is not None and len(si.on_wait) > maxw:
                waits = list(si.on_wait)
                extra, keep = waits[:-maxw], waits[-maxw:]
                for w in extra:
                    ev = mybir.InstEventSemaphore(
                        name=nc.get_next_instruction_name(), ins=[], outs=[])
                    ev.engine = ins.engine
                    ev.sync_info = mybir.SyncInfo(on_wait=[w], on_update=[])
                    nc.register_instruction(ev, overwrite=True)
                    out.append(ev)
                ins.sync_info = mybir.SyncInfo(
                    on_wait=keep, on_update=list(si.on_update))
                changed = True
            out.append(ins)
        if changed:
            bb.instructions = out


def build_nc():
    nc = bass.Bass(target_bir_lowering=False)
    x_d = nc.dram_tensor("x", [C, T], F32, kind="ExternalInput")
    xb16_d = nc.dram_tensor("xb16", [C, T], BF16, kind="ExternalInput")
    w1t_d = nc.dram_tensor("w1t", [3, NCH, P, K * NCH * P], BF16,
                           kind="ExternalInput")
    w2w_d = nc.dram_tensor("w2w", [3, NCH, P, 4 * NCH * P], BF16,
                           kind="ExternalInput")
    pp_d = nc.dram_tensor("pp", [NCH, P, NPARAM], F32, kind="ExternalInput")
    y_d = nc.dram_tensor("y", [C, T], F32, kind="ExternalOutput")

    with SplitDrainTileContext(nc) as tc:
        with (
            tc.tile_pool(name="persist", bufs=1) as p1,
            tc.tile_pool(name="wts", bufs=1) as pw,
            tc.tile_pool(name="px", bufs=2) as px,
            tc.tile_pool(name="pzz", bufs=2) as pz,
            tc.tile_pool(name="prr", bufs=2) as pr,
            tc.tile_pool(name="puu", bufs=2) as pu_,
            tc.tile_pool(name="pcb", bufs=1) as pcmb,
            tc.tile_pool(name="ps1", bufs=3, space="PSUM") as pps1,
            tc.tile_pool(name="psm", bufs=5, space="PSUM") as ppsm,
        ):
            xf = [p1.tile([P, T], F32, tag=f"xf{c}", name=f"xf{c}")
                  for c in range(NCH)]
            xb = [p1.tile([P, TPW], BF16, tag=f"xb{c}", name=f"xb{c}")
                  for c in range(NCH)]
            hE = [p1.tile([P, UPW], BF16, tag=f"hE{c}", name=f"hE{c}")
                  for c in range(NCH)]
            hO = [p1.tile([P, UPW], BF16, tag=f"hO{c}", name=f"hO{c}")
                  for c in range(NCH)]
            pt = [p1.tile([P, NPARAM], F32, tag=f"pt{c}", name=f"pt{c}")
                  for c in range(NCH)]

            def alloc_w1(i):
                return [pw.tile([P, K * NCH * P], BF16,
                                tag=f"w1_{c}", name=f"w1_{i}_{c}")
                        for c in range(NCH)]

            def alloc_w2(i):
                return [pw.tile([P, 4 * NCH * P], BF16,
                                tag=f"w2_{c}", name=f"w2_{i}_{c}")
                        for c in range(NCH)]

            for c in range(NCH):
                nc.sync.dma_start(out=pt[c], in_=pp_d[c])
                nc.vector.memset(xb[c][:, 0:PAD], 0.0)
                nc.vector.memset(xb[c][:, PAD + T:TPW], 0.0)
                nc.vector.memset(hE[c][:, 0:PADU], 0.0)
                nc.vector.memset(hE[c][:, PADU + U:UPW], 0.0)
                nc.vector.memset(hO[c][:, 0:PADU], 0.0)
                nc.vector.memset(hO[c][:, PADU + U:UPW], 0.0)

            # DMA queue order is the startup critical path. First conv1
            # matmuls need the co=0 strip of block-0 w1 plus xb jt0..1;
            # then the rest of w1; the rest of xb; block-0 winograd w2;
            # and last the f32 x (needed from the first conv2 combos,
            # ~50us in, chunk-by-chunk in co order).
            w1t0 = alloc_w1(0)
            CW = K * P
            for c in range(NCH):
                nc.sync.dma_start(out=w1t0[c][:, 0:CW],
                                  in_=w1t_d[0, c][:, 0:CW])
            for jt in range(2):
                for c in range(NCH):
                    sl = slice(jt * TT, (jt + 1) * TT)
                    nc.sync.dma_start(
                        out=xb[c][:, PAD + jt * TT:PAD + (jt + 1) * TT],
                        in_=xb16_d[c * P:(c + 1) * P, sl])
            for c in range(NCH):
                nc.sync.dma_start(out=w1t0[c][:, CW:],
                                  in_=w1t_d[0, c][:, CW:])
            for jt in range(2, NT):
                for c in range(NCH):
                    sl = slice(jt * TT, (jt + 1) * TT)
                    nc.sync.dma_start(
                        out=xb[c][:, PAD + jt * TT:PAD + (jt + 1) * TT],
                        in_=xb16_d[c * P:(c + 1) * P, sl])
            w2t0 = alloc_w2(0)
            for c in range(NCH):
                nc.sync.dma_start(out=w2t0[c], in_=w2w_d[0, c])
            for c in range(NCH):
                nc.sync.dma_start(out=xf[c], in_=x_d[c * P:(c + 1) * P, :])

            w1cur, w2cur = w1t0, w2t0
            state = {"w1": w1cur, "w2": w2cur}

            def emit_conv1_tile(i, jt, co, w1t):
                d = DILATIONS[i]
                base = i * 6
                zbap = pt[co][:, base + 0:base + 1]   # s2*b1
                zsap = pt[co][:, base + 1:base + 2]   # s1*s2
                raap = pt[co][:, base + 2:base + 3]   # s1*a/pi
                rbap = pt[co][:, base + 3:base + 4]   # b1*a/pi
                ibap = pt[co][:, base + 4:base + 5]   # invb*s2
                ps = pps1.tile([P, TT], F32, tag="ps1")
                col0 = PAD + jt * TT
                n = 0
                for ci in range(NCH):
                    for k in range(K):
                        sh = (k - 1) * d
                        nc.tensor.matmul(
                            ps,
                            w1t[ci][:, (co * K + k) * P:(co * K + k + 1) * P],
                            xb[ci][:, col0 + sh:col0 + sh + TT],
                            start=(n == 0), stop=(n == 11),
                        )
                        n += 1
                # z' = s2*(s1*psum + b1)  (s2 folded so conv2 weights stay
                # exact); kept bf16 -- it only feeds the bf16 h' planes.
                zt = pz.tile([P, TT], BF16, tag="z")
                nc.scalar.activation(zt, ps, AF.Identity,
                                     bias=zbap, scale=zsap)
                # r = a*z/pi; dd = mod(r,1) in place; u = sin(pi*dd);
                # sq = u^2 (ScalarE).
                rt = pr.tile([P, TT], F32, tag="r")
                nc.scalar.activation(rt, ps, AF.Identity,
                                     bias=rbap, scale=raap)
                # range-reduce r mod 1 (sin^2 is pi-periodic in pi*r):
                # ScalarE converts r to i32 (round), DVE subtracts in place.
                ri = pr.tile([P, TT], mybir.dt.int16, tag="ri")
                nc.scalar.activation(ri, rt, AF.Identity)
                nc.vector.tensor_tensor(rt, rt, ri, ALU.subtract)
                ut = pu_.tile([P, TT], F32, tag="u")
                nc.scalar.activation(ut, rt, AF.Sin, scale=float(np.pi))
                nc.scalar.activation(ut, ut, AF.Square)
                # h' = invb*s2*u^2 + z', split into E/O planes (bf16)
                pu0 = PADU + jt * (TT // 2)
                nc.vector.scalar_tensor_tensor(
                    hE[co][:, pu0:pu0 + TT // 2],
                    ut[:, 0:TT:2], ibap, zt[:, 0:TT:2],
                    ALU.mult, ALU.add)
                nc.vector.scalar_tensor_tensor(
                    hO[co][:, pu0:pu0 + TT // 2],
                    ut[:, 1:TT:2], ibap, zt[:, 1:TT:2],
                    ALU.mult, ALU.add)

            def emit_conv2_x(i, k):
                pu0 = PADU + k * UT
                xs = []
                for c in range(NCH):
                    E = hE[c][:, pu0:pu0 + UT]
                    En = hE[c][:, pu0 + 1:pu0 + UT + 1]
                    Om = hO[c][:, pu0 - 1:pu0 + UT - 1]
                    O = hO[c][:, pu0:pu0 + UT]
                    X0 = px.tile([P, UT], BF16, tag=f"X0_{c}",
                                 name=f"X0_{i}_{k}_{c}")
                    X1 = px.tile([P, UT], BF16, tag=f"X1_{c}",
                                 name=f"X1_{i}_{k}_{c}")
                    X2 = px.tile([P, UT], BF16, tag=f"X2_{c}",
                                 name=f"X2_{i}_{k}_{c}")
                    X3 = px.tile([P, UT], BF16, tag=f"X3_{c}",
                                 name=f"X3_{i}_{k}_{c}")
                    nc.vector.tensor_tensor(X0, Om, O, ALU.subtract)
                    nc.vector.tensor_tensor(X1, E, O, ALU.add)
                    nc.vector.tensor_tensor(X2, O, E, ALU.subtract)
                    nc.vector.tensor_tensor(X3, E, En, ALU.subtract)
                    xs.append((X0, X1, X2, X3))
                return xs

            def emit_conv2_group(i, k, co, xs, w2t):
                base = i * 6
                b2ap = pt[co][:, base + 5:base + 6]
                ms = [ppsm.tile([P, UT], F32, tag="psm",
                                name=f"m{j}_{i}_{k}_{co}")
                      for j in range(4)]
                for j in range(4):
                    for c in range(NCH):
                        nc.tensor.matmul(
                            ms[j],
                            w2t[c][:, (co * 4 + j) * P:(co * 4 + j + 1) * P],
                            xs[c][j],
                            start=(c == 0), stop=(c == NCH - 1),
                        )
                # y_e = m0+m1+m2+b2 ; y_o = m1-m2-m3+b2 ; x += y
                # (DVE may read at most one PSUM operand per instruction,
                # so ScalarE first stages m1+b2 into SBUF.)
                m1c = pcmb.tile([P, UT], F32, tag="m1c")
                nc.scalar.activation(m1c, ms[1], AF.Identity, bias=b2ap)
                t1 = pcmb.tile([P, UT], F32, tag="t1")
                nc.vector.tensor_tensor(t1, m1c, ms[2], ALU.add)
                t2 = pcmb.tile([P, UT], F32, tag="t2")
                nc.vector.tensor_tensor(t2, m1c, ms[2], ALU.subtract)
                ve = pcmb.tile([P, UT], F32, tag="m1c", name=f"ve_{i}_{k}_{co}")
                nc.vector.tensor_tensor(ve, t1, ms[0], ALU.add)
                vo = pcmb.tile([P, UT], F32, tag="t1", name=f"vo_{i}_{k}_{co}")
                nc.vector.tensor_tensor(vo, t2, ms[3], ALU.subtract)
                t0c = k * 2 * UT
                xe = xf[co][:, t0c + 0:t0c + 2 * UT:2]
                xo = xf[co][:, t0c + 1:t0c + 2 * UT:2]
                nc.vector.tensor_tensor(xe, xe, ve, ALU.add)
                nc.vector.tensor_tensor(xo, xo, vo, ALU.add)
                if i < 2:
                    nc.vector.tensor_copy(
                        xb[co][:, PAD + t0c:PAD + t0c + 2 * UT],
                        xf[co][:, t0c:t0c + 2 * UT])
                else:
                    nc.sync.dma_start(
                        out=y_d[co * P:(co + 1) * P, t0c:t0c + 2 * UT],
                        in_=xf[co][:, t0c:t0c + 2 * UT])

            # --- emission schedule -------------------------------------
            # conv2 group (i,k) is ready once conv1(i, jt<=min(2k+2,7))
            # are all emitted; conv1(i+1, jt) requires conv2(i, jt//2)
            # emitted (xb updated). Interleave one conv2 co-group per
            # conv1 tile once the pipe is primed, so the in-order PE and
            # DVE queues stay fed.
            pending = []          # list of (i, k, xs, w2t, next_co)
            conv2_done = {}       # (i,k) -> True once all 4 co emitted

            def enqueue_ready(i, jt_done, w2t):
                for k in range(NU):
                    need = min(2 * k + 2, NT - 1)
                    if jt_done == need:
                        xs = emit_conv2_x(i, k)
                        pending.append([i, k, xs, w2t])

            def pop_one():
                if not pending:
                    return
                item = pending[0]
                i, k, xs, w2t = item
                co = conv2_done.get((i, k), 0)
                emit_conv2_group(i, k, co, xs, w2t)
                conv2_done[(i, k)] = co + 1
                if co + 1 == NCH:
                    pending.pop(0)

            def flush_through(i, k):
                while any(p[0] == i and p[1] <= k for p in pending):
                    pop_one()

            for i in range(3):
                w1t, w2t = state["w1"], state["w2"]
                for jt in range(NT):
                    if i > 0:
                        flush_through(i - 1, jt // 2)
                    for co in range(NCH):
                        emit_conv1_tile(i, jt, co, w1t)
                        if co % 2 == 1:
                            pop_one()
                    enqueue_ready(i, jt, w2t)
                # prefetch next block weights (tag reuse: DMA waits on
                # last readers of the current block's tiles)
                if i < 2:
                    w1n = alloc_w1(i + 1)
                    for c in range(NCH):
                        nc.sync.dma_start(out=w1n[c], in_=w1t_d[i + 1, c])
                    w2n = alloc_w2(i + 1)
                    for c in range(NCH):
                        nc.sync.dma_start(out=w2n[c], in_=w2w_d[i + 1, c])
                    state["w1"], state["w2"] = w1n, w2n
            while pending:
                pop_one()
    _split_sync_waits(nc)
    return nc


_NC = None


def _get_nc():
    global _NC
    if _NC is None:
        _NC = build_nc()
    return _NC


def _host_params(w1, b1, alpha, beta, w2, b2):
    """Ternarize weights, build winograd conv2 weights, and fold snake
    and scale params, matching the reference's jax-on-CPU f32 numerics."""
    import jax
    import jax.numpy as jnp

    cpu = jax.devices("cpu")[0]

    w1t = np.empty((3, NCH, P, K * NCH * P), dtype=ml_dtypes.bfloat16)
    w2w = np.empty((3, NCH, P, 4 * NCH * P), dtype=ml_dtypes.bfloat16)
    pp = np.zeros((NCH, P, NPARAM), dtype=np.float32)
    pi = np.float32(np.pi)

    with jax.default_device(cpu):
        for i in range(3):
            svals = []
            terns = []
            for w in (w1[i], w2[i]):
                s = jnp.mean(jnp.abs(w))
                tern = jnp.clip(jnp.round(w / (s + EPS_Q)), -1.0, 1.0)
                svals.append(np.float32(s))
                terns.append(np.asarray(tern, dtype=np.float32))
            s1, s2 = svals
            t1, t2 = terns
            # conv1: [co, ci, k] -> [cich, ci_in, coch, k, co_in]
            # (co-major free dim so a single co chunk is one contiguous DMA)
            t5 = t1.reshape(NCH, P, NCH, P, K).transpose(2, 3, 0, 4, 1)
            w1t[i] = t5.reshape(NCH, P, K * NCH * P).astype(
                ml_dtypes.bfloat16)
            # conv2 winograd point weights (exact in bf16: ternary sums
            # and halves)
            g0, g1, g2 = t2[:, :, 0], t2[:, :, 1], t2[:, :, 2]
            Wj = np.stack([g0,
                           0.5 * (g0 + g1 + g2),
                           0.5 * (g0 - g1 + g2),
                           g2], axis=0)  # [4, co, ci]
            # -> [cich, ci_in, coch, j, co_in]
            t5 = Wj.reshape(4, NCH, P, NCH, P).transpose(3, 4, 1, 0, 2)
            w2w[i] = t5.reshape(NCH, P, 4 * NCH * P).astype(
                ml_dtypes.bfloat16)

            a = np.asarray(jnp.exp(alpha[i]), dtype=np.float32)
            bsn = np.asarray(jnp.exp(beta[i]), dtype=np.float32)
            invb = np.asarray(
                jnp.float32(1.0) / (jnp.asarray(bsn) + jnp.float32(EPS_SNAKE)),
                dtype=np.float32)
            base = i * 6
            pp[:, :, base + 0] = (s2 * b1[i]).reshape(NCH, P)
            pp[:, :, base + 1] = s1 * s2
            pp[:, :, base + 2] = (s1 * a / pi).reshape(NCH, P)
            pp[:, :, base + 3] = (b1[i] * a / pi).reshape(NCH, P)
            pp[:, :, base + 4] = (invb * s2).reshape(NCH, P)
            pp[:, :, base + 5] = b2[i].reshape(NCH, P)
    return w1t, w2w, pp


def sim_feed(sim, np_inputs, b=0):
    """Feed CoreSim tensors for batch element b (test harness helper)."""
    w1t, w2w, pp = _host_params(
        np_inputs["w1"], np_inputs["b1"], np_inputs["alpha"],
        np_inputs["beta"], np_inputs["w2"], np_inputs["b2"])
    x = np.asarray(np_inputs["x"], dtype=np.float32)
    sim.tensor("x")[:] = x[b]
    sim.tensor("xb16")[:] = x[b].astype(ml_dtypes.bfloat16)
    sim.tensor("w1t")[:] = w1t
    sim.tensor("w2w")[:] = w2w
    sim.tensor("pp")[:] = pp


def kernel(x, w1, b1, alpha, beta, w2, b2):
    global LAST_EXEC_NS
    x = np.asarray(x, dtype=np.float32)
    w1 = np.asarray(w1, dtype=np.float32)
    b1 = np.asarray(b1, dtype=np.float32)
    alpha = np.asarray(alpha, dtype=np.float32)
    beta = np.asarray(beta, dtype=np.float32)
    w2 = np.asarray(w2, dtype=np.float32)
    b2 = np.asarray(b2, dtype=np.float32)

    w1t, w2w, pp = _host_params(w1, b1, alpha, beta, w2, b2)
    nc = _get_nc()

    in_maps = [
        {"x": x[b], "xb16": x[b].astype(ml_dtypes.bfloat16),
         "w1t": w1t, "w2w": w2w, "pp": pp}
        for b in range(B)
    ]
    res = run_bass_kernel_spmd(
        nc, in_maps, core_ids=list(range(B)), trace=TRACE)
    LAST_EXEC_NS = res.exec_time_ns
    global LAST_RESULT
    LAST_RESULT = res

    out = np.stack([res.results[b]["y"] for b in range(B)], axis=0)
    return out.astype(np.float32)
